# revision 11
# baseline (speedup 1.0000x reference)
"""Trainium2 Bass kernel for CommunityPassing (segment mean + gather).

Algorithm (8 NeuronCores, data-parallel over nodes):
  host: relabel communities so table ids are sorted by descending padded
        per-core max count (pad = ceil4(max over cores)); shard x/community
        over 8 cores along the node axis; within each shard, stably sort
        node indices by table id and pack them into 128-row tiles grouped
        by community "chunk" (128 table ids per chunk). Pad each
        (core, chunk) block to a shared tile count so all cores run one
        SPMD program. x is sent as fp16 in a (p t) f layout so every DMA
        line is contiguous per partition.
  dev:  phase 1 - stream sorted x tiles; build a per-tile one-hot
        selection matrix B[node, local_comm] with a DVE is_equal against
        an iota row; matmul B^T @ x_tile (fp16 x fp16 -> fp32 PSUM)
        accumulating per community chunk -> per-core partial sums.
        Scale each chunk's partial sums by host-computed 1/count (fp16
        out) BEFORE the collective: mean = sum_m (s_m / cnt), so an
        fp16 AllReduce over the scaled partials yields the mean table
        directly. 4 pipelined AllReduces of 2 chunks each (64KB fp16).
        Load each AR result back with a 4x stride-0 broadcast into a
        duplicated SBUF mean table (rows = [mean_c x4] for 2KB DMA lines).
        phase 2 - no compute: for each host-planned run of consecutive
        table ids with equal pad, ONE broadcast DMA with a stride-0
        source AP fans each community's duplicated mean row out to its
        padded, contiguous block of output rows (fp16).
  host: gather the per-node rows from the padded community blocks,
        un-permute, upcast fp16 -> fp32, concatenate the 8 shards.
"""

import os
import sys

import numpy as np

for _p in ("/opt/trn_rl_repo", "/opt/pypackages"):
    if _p not in sys.path and os.path.isdir(_p):
        sys.path.append(_p)

# Problem constants (hardcoded per the task contract).
N_FULL = 500000
F = 256
NUM_COMMS = 1000
EPS = 1e-12
M = 8               # cores
P = 128             # partitions
NC_CHUNKS = 8       # community chunks of 128 (8*128 = 1024 >= 1000)
XB = 8              # x tiles per streaming in-DMA (8 * 512B/part = 4KB lines)
DUP = 4             # mean-row duplication factor (-> 2KB broadcast lines)
AR_GROUPS = [2, 2, 2, 2]  # chunks per AllReduce
GAP = 4             # skipped rows between community blocks (AP un-merge)
RUN_W = 128         # communities per broadcast run (full chunk: 16-engine spread)

# Stash of the most recent run's BassKernelResults (for test harnesses).
LAST_RESULTS = None


def _host_prep(x, community):
    """Build per-core device inputs. Returns (in_maps, plan)."""
    x = np.asarray(x)
    community = np.asarray(community).astype(np.int64)
    n = x.shape[0]
    assert n % M == 0
    nl = n // M

    comm_sh = community.reshape(M, nl)

    # per-(core, community) counts and the shared padded block size
    cnts_mc = np.zeros((M, NUM_COMMS), dtype=np.int64)
    for m in range(M):
        cnts_mc[m] = np.bincount(comm_sh[m], minlength=NUM_COMMS)
    max_c = cnts_mc.max(axis=0)
    pad_c = DUP * ((max_c + DUP - 1) // DUP)  # multiple of DUP

    # relabel: table id = rank of community ordered by descending pad
    order = np.argsort(-pad_c, kind="stable")      # community ids, big first
    table_of_comm = np.empty(NUM_COMMS, dtype=np.int64)
    table_of_comm[order] = np.arange(NUM_COMMS)
    pad_t = pad_c[order]                           # pad by table id

    # one broadcast run per chunk: uniform g = chunk max pad, plus GAP gap
    # rows between community blocks so the destination AP cannot collapse
    # to 2D (keeps the outer dim = communities -> even 16-engine spread)
    runs = []  # (table_id0, n_comms, g, stride_rows)
    blk_start = np.zeros(NUM_COMMS + 1, dtype=np.int64)
    base = 0
    for lo in range(0, NUM_COMMS, RUN_W):
        hi = min(lo + RUN_W, NUM_COMMS)
        ncomm = int((pad_t[lo:hi] > 0).sum())
        if ncomm == 0:
            continue
        g = int(pad_t[lo])                      # sorted desc -> max of run
        stride = g + GAP
        runs.append((int(lo), ncomm, g, stride))
        blk_start[lo : lo + ncomm] = base + np.arange(ncomm) * stride
        base += ncomm * stride
    out_rows = int(base)

    tid_sh = table_of_comm[comm_sh]                # [M, nl] table ids
    perms = np.argsort(tid_sh, axis=1, kind="stable")
    tid_sorted = np.take_along_axis(tid_sh, perms, axis=1)

    # per (core, chunk) node counts -> shared tile counts
    chunk_ids = tid_sorted >> 7
    cnts = np.zeros((M, NC_CHUNKS), dtype=np.int64)
    for m in range(M):
        cnts[m] = np.bincount(chunk_ids[m], minlength=NC_CHUNKS)[:NC_CHUNKS]
    t_k = np.maximum(1, -(-cnts.max(axis=0) // P))  # ceil, shared by all cores
    t_total = int(t_k.sum())
    tile_off = np.concatenate([[0], np.cumsum(t_k)])

    # counts -> 1/max(cnt, eps) in [p, k] table layout (table id = k*128 + p)
    cnt_full = cnts_mc.sum(axis=0).astype(np.float64)
    inv_by_table = np.zeros((NC_CHUNKS * P,), np.float32)
    inv_by_table[:NUM_COMMS] = (1.0 / np.maximum(cnt_full[order], EPS)).astype(
        np.float32
    )
    invc = np.ascontiguousarray(inv_by_table.reshape(NC_CHUNKS, P).T)  # [128, 8]

    iota16 = np.ascontiguousarray(
        np.tile(np.arange(P, dtype=np.float16), (P, XB))
    )  # [128, XB*128], each row = 0..127 repeated XB times

    in_maps = []
    sort_info = []
    for m in range(M):
        x_m = np.asarray(x[m * nl : (m + 1) * nl], dtype=np.float16)
        xs = np.zeros((t_total * P, F), dtype=np.float16)   # (p t) f rows
        locid = np.full((P, t_total), -1.0, dtype=np.float16)
        start = 0
        for k in range(NC_CHUNKS):
            c = int(cnts[m, k])
            j = np.arange(c)
            tt = tile_off[k] + (j >> 7)
            pp = j & 127
            rows = pp * t_total + tt
            sel = perms[m, start : start + c]
            xs[rows] = x_m[sel]
            locid[pp, tt] = tid_sorted[m, start : start + c] - (k << 7)
            start += c
        in_maps.append(
            {"xs": xs, "locid": locid, "iota": iota16, "invc": invc}
        )
        sort_info.append((perms[m], tid_sorted[m]))

    plan = {
        "nl": nl,
        "t_k": [int(v) for v in t_k],
        "t_total": t_total,
        "tile_off": [int(v) for v in tile_off],
        "runs": runs,
        "blk_start": blk_start,
        "out_rows": out_rows,
        "sort_info": sort_info,
    }
    return in_maps, plan


def _build_program(plan, use_collective=True):
    from concourse import bacc, mybir, tile

    t_total = plan["t_total"]
    tile_off = plan["tile_off"]
    runs = plan["runs"]
    blk_start = plan["blk_start"]
    out_rows = plan["out_rows"]

    dt = mybir.dt
    nc = bacc.Bacc("TRN2", target_bir_lowering=False, debug=False, num_devices=M)

    xs = nc.dram_tensor("xs", [t_total * P, F], dt.float16, kind="ExternalInput")
    locid = nc.dram_tensor("locid", [P, t_total], dt.float16, kind="ExternalInput")
    iota = nc.dram_tensor("iota", [P, XB * P], dt.float16, kind="ExternalInput")
    invc = nc.dram_tensor("invc", [P, NC_CHUNKS], dt.float32, kind="ExternalInput")
    out = nc.dram_tensor("out", [out_rows, F], dt.float16, kind="ExternalOutput")

    xs_view = xs.ap().rearrange("(p t) f -> p t f", p=P)  # [128, T, 256]
    n_ar = len(AR_GROUPS)
    group_of_chunk = []
    for a, sz in enumerate(AR_GROUPS):
        group_of_chunk += [a] * sz
    group_base = [sum(AR_GROUPS[:a]) for a in range(n_ar)]

    with tile.TileContext(nc) as tc:
        with (
            tc.tile_pool(name="const", bufs=1) as constp,
            tc.tile_pool(name="xsp", bufs=8) as xsp,
            tc.tile_pool(name="bp", bufs=3) as bp,
            tc.tile_pool(name="s16", bufs=2) as s16p,
            tc.tile_pool(name="psum", bufs=2, space="PSUM") as psp,
            tc.tile_pool(name="dram", bufs=1, space="DRAM") as dramp,
        ):
            iota_t = constp.tile([P, XB * P], dt.float16)
            nc.sync.dma_start(out=iota_t[:], in_=iota.ap())
            locid_t = constp.tile([P, t_total], dt.float16)
            nc.sync.dma_start(out=locid_t[:], in_=locid.ap())
            invc_t = constp.tile([P, NC_CHUNKS], dt.float32)
            nc.sync.dma_start(out=invc_t[:], in_=invc.ap())

            # duplicated mean table: row c of chunk k = [mean_c] * DUP (fp16)
            dup_t = constp.tile([P, NC_CHUNKS * DUP * F], dt.float16)

            # dummy collective fired immediately: pulls the one-time NRT
            # CC-stream init barrier forward so real ARs are not gated on it
            if use_collective:
                warm_in = nc.dram_tensor("warm_in", [P, 2], dt.float16,
                                         kind="Internal")
                warm_out = nc.dram_tensor("warm_out", [P, 2], dt.float16,
                                          kind="Internal", addr_space="Shared")
                warm_sb = constp.tile([P, 2], dt.float16)
                nc.vector.memset(warm_sb[:], 0.0)
                nc.sync.dma_start(out=warm_in.ap(), in_=warm_sb[:])
                nc.gpsimd.collective_compute(
                    "AllReduce",
                    mybir.AluOpType.add,
                    replica_groups=[list(range(M))],
                    ins=[warm_in.ap()],
                    outs=[warm_out.ap()],
                )

            ar_ins = [
                nc.dram_tensor(
                    f"ar_in{a}", [P, AR_GROUPS[a] * F], dt.float16, kind="Internal"
                )
                for a in range(n_ar)
            ]
            ar_outs = [
                nc.dram_tensor(
                    f"ar_out{a}", [P, AR_GROUPS[a] * F], dt.float16,
                    kind="Internal", addr_space="Shared",
                )
                for a in range(n_ar)
            ]

            s16_t = None
            xsb = None
            b8 = None
            for k in range(NC_CHUNKS):
                a = group_of_chunk[k]
                gsz = AR_GROUPS[a]
                if k == group_base[a]:
                    s16_t = s16p.tile([P, gsz * F], dt.float16, tag="s16")
                # ---- phase 1: streamed one-hot matmul partial sums ----
                psum_t = psp.tile([P, F], dt.float32)
                for t in range(tile_off[k], tile_off[k + 1]):
                    if t % XB == 0:
                        w = min(XB, t_total - t)
                        xsb = xsp.tile([P, XB * F], dt.float16, tag="xsb")
                        nc.sync.dma_start(
                            out=xsb[:, : w * F].rearrange("p (b f) -> p b f", b=w),
                            in_=xs_view[:, t : t + w, :],
                        )
                        # one DVE op builds B for all w tiles of this group
                        b8 = bp.tile([P, XB * P], dt.float16, tag="b8")
                        nc.vector.tensor_tensor(
                            out=b8[:, : w * P],
                            in0=iota_t[:, : w * P],
                            in1=locid_t[:, t : t + w]
                            .unsqueeze(2)
                            .to_broadcast((P, w, P)),
                            op=mybir.AluOpType.is_equal,
                        )
                    j = t % XB
                    nc.tensor.matmul(
                        psum_t[:],
                        lhsT=b8[:, j * P : (j + 1) * P],
                        rhs=xsb[:, j * F : (j + 1) * F],
                        start=(t == tile_off[k]),
                        stop=(t == tile_off[k + 1] - 1),
                    )
                # scaled partial means (fp16) for this chunk
                kk_in_g = k - group_base[a]
                nc.vector.tensor_scalar(
                    s16_t[:, kk_in_g * F : (kk_in_g + 1) * F],
                    psum_t[:],
                    invc_t[:, k : k + 1],
                    None,
                    mybir.AluOpType.mult,
                )
                if kk_in_g == gsz - 1:
                    nc.sync.dma_start(out=ar_ins[a].ap(), in_=s16_t[:])
                    if use_collective:
                        nc.gpsimd.collective_compute(
                            "AllReduce",
                            mybir.AluOpType.add,
                            replica_groups=[list(range(M))],
                            ins=[ar_ins[a].ap()],
                            outs=[ar_outs[a].ap()],
                        )
                        ar_res = ar_outs[a]
                    else:
                        nc.sync.dma_start(out=ar_outs[a].ap(), in_=ar_ins[a].ap())
                        ar_res = ar_outs[a]
                    # load back with 4x duplication (stride-0 DRAM source)
                    src = (
                        ar_res.ap()
                        .rearrange("p (c f) -> p c f", f=F)
                        .unsqueeze(2)
                        .to_broadcast((P, gsz, DUP, F))
                    )
                    base = group_base[a] * DUP * F
                    nc.sync.dma_start(
                        out=dup_t[:, base : base + gsz * F * DUP].rearrange(
                            "p (c d f) -> p c d f", d=DUP, f=F
                        ),
                        in_=src,
                    )
                    # ---- phase 2: broadcast runs for the chunks just ARed ----
                    for (tid0, ncomm, g, stride) in runs:
                        kk = tid0 >> 7
                        if not (group_base[a] <= kk <= k):
                            continue
                        p0 = tid0 & 127
                        reps = g // DUP
                        src_run = (
                            dup_t[
                                p0 : p0 + ncomm,
                                kk * DUP * F : (kk + 1) * DUP * F,
                            ]
                            .unsqueeze(1)
                            .to_broadcast((ncomm, reps, DUP * F))
                        )
                        r0 = int(blk_start[tid0])
                        dst = (
                            out.ap()[r0 : r0 + ncomm * stride, :]
                            .rearrange("(c s) f -> c s f", s=stride)[:, :g, :]
                            .rearrange("c (r d) f -> c r (d f)", d=DUP)
                        )
                        nc.scalar.dma_start(out=dst, in_=src_run)

    nc.compile()
    return nc


def kernel(x, community):
    global LAST_RESULTS
    from concourse.bass_utils import run_bass_kernel_spmd

    in_maps, plan = _host_prep(x, community)
    nc = _build_program(plan)
    res = run_bass_kernel_spmd(nc, in_maps, core_ids=list(range(M)))
    LAST_RESULTS = res
    nl = plan["nl"]
    blk_start = plan["blk_start"]
    outs = []
    for m in range(M):
        out_dev = res.results[m]["out"]  # [out_rows, 256] fp16
        perm, tid_sorted = plan["sort_info"][m]
        starts = np.searchsorted(tid_sorted, np.arange(NUM_COMMS))
        rank = np.arange(nl) - starts[tid_sorted]
        rows = blk_start[tid_sorted] + rank
        out_m = np.empty((nl, F), dtype=np.float32)
        out_m[perm] = out_dev[rows].astype(np.float32)
        outs.append(out_m)
    return np.concatenate(outs, axis=0)
